# revision 25
# baseline (speedup 1.0000x reference)
"""Trainium2 kernel for nn_AlignmentLayer.

y[l] = (x[l] - x_c[l]) @ R[l]  for l in 0..8191, x[l] is [2000, 3].

Host side computes the per-frame 3x3 rotation R[l] (Kabsch via SVD of the
64-atom cross-covariance) and translation t[l] = -x_c[l] @ R[l] -- tiny
O(L*64) work.  The device kernel does the memory-bound part: stream all of
x through SBUF and apply the per-frame affine map.

Device layout (per core, 1024 frames, data-parallel over frames):
  - frames on SBUF partitions, 128 per block, 8 blocks per core
  - each DRAM row = [12 params || 6000 coords] so one DMA per block brings
    both; params cols 0..8 = R row-major, 9..11 = t
  - compute is in-place on the x tile: for each output coord b,
      y_b = ((x_a0 * R[0,b] + t_b) + x_a1*R[1,b]) + x_a2*R[2,b]
    via tensor_scalar + 2x scalar_tensor_tensor with per-partition scalars
    and stride-3 access patterns (no deinterleave, no extra y tile)
  - raw bass with manual semaphores: SP issues all DMAs on the HWDGE FIFO
    ring, DVE does all compute; standalone wait_ge instructions only
    (this walrus build allows at most ONE attached sem wait per instruction,
    which Tile's scheduler cannot guarantee for this DMA pattern)
"""

from contextlib import ExitStack

import numpy as np

import concourse.bass as bass
import concourse.mybir as mybir
from concourse.bass_utils import run_bass_kernel_spmd

L, N, NR = 8192, 2000, 64
N_CORES = 8
L_PER_CORE = L // N_CORES          # 1024
BLOCKS = L_PER_CORE // 128         # 8
ROW = 12 + 3 * N                   # params + coords per frame
F32 = mybir.dt.float32


def _build_nc(reps=1):
    """reps > 1 replays the whole pipeline (same data) for HW timing runs;
    all semaphore values are linear in the global block counter G.

    DMA completion sems are LANED (7 in-lanes, 4 out-lanes): concurrent DMAs
    on one ring can deliver their sem updates out of order (each update is
    the last descriptor on ONE of the 16 SDMA engine rings, and engine skew
    reorders them), so a single counting sem is racy.  Within a lane,
    consecutive DMAs are ordered by a trigger-side wait on the lane's prior
    value, which is always already satisfied by the slot-reuse gating."""
    nc = bass.Bass()
    x = nc.declare_dram_parameter("x", [L_PER_CORE, ROW], F32, isOutput=False)
    y = nc.declare_dram_parameter("y", [L_PER_CORE, 3 * N], F32, isOutput=True)

    mult = mybir.AluOpType.mult
    add = mybir.AluOpType.add
    ident = mybir.ActivationFunctionType.Identity
    S = 6   # x-tile slots
    LI = 7  # s_in lanes (> max concurrent in-DMAs)
    LO = 4  # s_out lanes (> max concurrent out-DMAs)
    TOT = BLOCKS * reps

    with (
        ExitStack() as ctx,
        nc.sbuf_tensor([128, S * ROW], F32) as xts,
        nc.sbuf_tensor([128, 6 * N], F32) as tts,
        nc.semaphore("s_act") as s_act,
        nc.semaphore("s_dve") as s_dve,
        nc.Block() as block,
    ):
        s_in = [ctx.enter_context(nc.semaphore(f"s_in{i}")) for i in range(LI)]
        s_out = [ctx.enter_context(nc.semaphore(f"s_out{i}")) for i in range(LO)]
        # two sets of three t tiles, ping-ponged between ACT (producer) and
        # DVE (consumer) across blocks
        tset = [[tts[:, (3 * s + b) * N:(3 * s + b + 1) * N] for b in range(3)]
                for s in range(2)]

        def slot_ap(slot):
            return xts[:, slot * ROW:(slot + 1) * ROW]

        # NOTE: pairing blocks into 6 MB DMAs was tested and LOSES (~+0.4us):
        # the model's per-DMA overhead scales with descriptor rows (the 3D AP
        # doubles partition-chunks), so merging saves nothing and coarsens
        # the completion signals.  Keep single-block DMAs.
        paired = False

        def in_done(eng, G):
            # block G's input landed (consumers only touch block G's slot)
            if paired:
                p = G // 2
                eng.wait_ge(s_in[p % LI], 16 * (p // LI + 1))
            else:
                eng.wait_ge(s_in[G % LI], 16 * (G // LI + 1))

        def out_dma(eng, M):
            eng.wait_ge(s_dve, M + 1)
            if M >= LO:
                # lane-order: our lane's previous tenant must have fired its
                # sem before ours can (concurrent completions reorder)
                eng.wait_ge(s_out[M % LO], 16 * (M // LO))
            blk = M % BLOCKS
            eng.dma_start(
                out=y[blk * 128:(blk + 1) * 128, :],
                in_=xts[:, (M % S) * ROW + 12:(M % S + 1) * ROW],
            ).then_inc(s_out[M % LO], 16)

        @block.sync
        def _(sync):
            # ins only -- the SP HWDGE ring streams input blocks, gated by
            # slot-free (out complete; cross-ring so a sem is required)
            if paired:
                for p in range(TOT // 2):
                    blk = 2 * p
                    s0 = blk % S
                    for tenant in range(max(0, blk - S), max(0, blk + 2 - S)):
                        # pair overwrites slots of blocks tenant..: their
                        # outs must have completed
                        sync.wait_ge(s_out[tenant % LO], 16 * (tenant // LO + 1))
                    sync.dma_start(
                        out=xts[:, s0 * ROW:(s0 + 2) * ROW]
                            .rearrange("p (s r) -> p s r", s=2),
                        in_=x[blk * 128:(blk + 2) * 128, :]
                            .rearrange("(s p) r -> p s r", s=2),
                    ).then_inc(s_in[p % LI], 16)
            else:
                for G in range(TOT):
                    if G >= S:
                        M = G - S
                        sync.wait_ge(s_out[M % LO], 16 * (M // LO + 1))
                    if G >= LI:
                        # lane-order (see out_dma); instant by slot gating
                        sync.wait_ge(s_in[G % LI], 16 * (G // LI))
                    blk = G % BLOCKS
                    sync.dma_start(
                        out=slot_ap(G % S),
                        in_=x[blk * 128:(blk + 1) * 128, :],
                    ).then_inc(s_in[G % LI], 16)
            # quiesce + reset: hardware semaphore values persist across NEFF
            # executions, and a rerun with stale counts sails through its
            # waits and races.  Two phases so most clears overlap the final
            # out transfers:
            #  1) s_dve>=TOT proves DVE(7) done, which proves every waiter of
            #     the s_in lanes and s_act has executed -> clear those now
            sync.wait_ge(s_act, TOT)
            sync.wait_ge(s_dve, TOT)
            n_in = TOT // 2 if paired else TOT
            for lane in range(LI):
                cnt = len(range(lane, n_in, LI))
                if cnt:
                    sync.wait_ge(s_in[lane], 16 * cnt)
                sync.sem_clear(s_in[lane])
            sync.sem_clear(s_act)
            #  2) s_out lanes + s_dve must wait for the final out completions
            #     (ACT's last trigger waits on s_dve; its execution is only
            #     proven by out(TOT-1)'s completion inc)
            for lane in range(LO):
                cnt = len(range(lane, TOT, LO))
                if cnt:
                    sync.wait_ge(s_out[lane], 16 * cnt)
            for sem in (*s_out, s_dve):
                sync.sem_clear(sem)

        @block.scalar
        def _(scalar):
            # ACT computes the chain heads and issues the DMA-outs on its own
            # HWDGE ring (decoupled from the in-ring)
            for G in range(TOT):
                in_done(scalar, G)
                if G >= 2:
                    # t-set reuse: DVE must be done with block G-2
                    scalar.wait_ge(s_dve, G - 1)
                xt = slot_ap(G % S)
                rt = xt[:, 0:12]
                xv = xt[:, 12:].rearrange("p (n a) -> p a n", a=3)
                ts = tset[G % 2]
                for b in range(3):
                    inst = nc.scalar.activation(
                        out=ts[b][:], in_=xv[:, 0, :], func=ident,
                        bias=rt[:, 9 + b:10 + b], scale=rt[:, b:b + 1])
                inst.then_inc(s_act, 1)
                if G >= 1:
                    out_dma(scalar, G - 1)
            out_dma(scalar, TOT - 1)

        @block.vector
        def _(vector):
            for G in range(TOT):
                in_done(vector, G)
                vector.wait_ge(s_act, G + 1)
                xt = slot_ap(G % S)
                rt = xt[:, 0:12]
                xv = xt[:, 12:].rearrange("p (n a) -> p a n", a=3)
                ts = tset[G % 2]
                for b in range(3):
                    # in-place: t tile goes t0 -> t1
                    nc.vector.scalar_tensor_tensor(
                        out=ts[b][:], in0=xv[:, 1, :], scalar=rt[:, 3 + b:4 + b],
                        in1=ts[b][:], op0=mult, op1=add)
                for b in range(3):
                    inst = nc.vector.scalar_tensor_tensor(
                        out=xv[:, b, :], in0=xv[:, 2, :], scalar=rt[:, 6 + b:7 + b],
                        in1=ts[b][:], op0=mult, op1=add)
                inst.then_inc(s_dve, 1)
    return nc


def _host_params(x, ref_x, align_atom_indices):
    """Per-frame rotation+translation, float64 for stability -> f32."""
    idx = np.asarray(align_atom_indices).astype(np.int64)
    ref0 = np.asarray(ref_x, np.float64)
    ref0 = ref0 - ref0.mean(axis=0)
    sel = np.asarray(x[:, idx, :], np.float64)          # [L, NR, 3]
    xc = sel.mean(axis=1)                               # [L, 3]
    xn = sel - xc[:, None, :]
    prod = np.einsum("lna,nb->lab", xn, ref0)           # [L, 3, 3]
    u, s, vh = np.linalg.svd(prod)
    det = np.linalg.det(u @ vh)
    d = np.ones_like(s)
    d[:, 2] = np.sign(det)
    R = np.einsum("lij,lj,ljk->lik", u, d, vh)          # [L, 3, 3]
    t = -np.einsum("la,lab->lb", xc, R)                 # [L, 3]
    return np.concatenate([R.reshape(L, 9), t], axis=1).astype(np.float32)


def run(x, ref_x, align_atom_indices, trace=False):
    params = _host_params(x, ref_x, align_atom_indices)          # [L, 12]
    xf = np.asarray(x, np.float32).reshape(L, 3 * N)
    packed = np.concatenate([params, xf], axis=1)                # [L, ROW]
    packed = np.ascontiguousarray(packed.reshape(N_CORES, L_PER_CORE, ROW))
    # rebuild per call: cheap (~1s), and keeps each run's module pristine
    # (bass2jax lowering touches the module; the end-of-program sem_clear in
    # _build_nc is what makes same-process reruns safe on the device side)
    nc = _build_nc()
    in_maps = [{"x": packed[i]} for i in range(N_CORES)]
    res = run_bass_kernel_spmd(nc, in_maps, core_ids=list(range(N_CORES)), trace=trace)
    out = np.concatenate([r["y"].reshape(L_PER_CORE, N, 3) for r in res.results], axis=0)
    return out, res.exec_time_ns


def kernel(x, ref_x, align_atom_indices):
    out, _ = run(x, ref_x, align_atom_indices)
    return out
